# revision 17
# baseline (speedup 1.0000x reference)
"""Multi-head self-attention kernel for Trainium2 (8 NeuronCores).

Problem: q,k,v [4000, 4096] fp32; the module attends q against itself
(k and v are ignored by the reference). 32 heads of dim 128.

Sharding: tensor-parallel over heads — each of the 8 cores owns 4 heads
(a [4000, 512] column slice of q) and computes its full attention
independently; the host concatenates the per-core outputs (the
"all-gather" along the feature axis).

Per-core algorithm (per head h):
  - load q_nat[kb] = q rows in natural layout (k on partitions)
  - build qT [hd=128 part, 4096] via PE transposes (pad q cols = 0)
  - for each q-superchunk (1024 wide) and each k-block kb (128 rows):
      mm1 (fp32r, N=512 x2): S_T[k, q] = qT[:,kb].T @ qT[:,qchunk]
      exp on ACT: pT = exp(S_T * scale)  (bf16)
      row-sums of pT on DVE (4x-mode tensor_scalar accumulate): because S
        is symmetric, the row sum over q of exp(S[k, q]) IS the softmax
        denominator l[k].
      mm2 (bf16, N=512 x2): O_T[hd, q] += v[kb].T @ pT  (accumulate
        over kb in PSUM)
  - l[kb] = sum of per-chunk partial sums ; r = 1/l
  - PE-transpose O_T back to [q, hd], multiply by r (per-partition), DMA out.

fp32r (TF32-like) runs the PE at 1 cycle/row for free dim >= 256, 4x
faster than plain fp32 matmul. The next head's qT build is emitted before
the previous head's epilogue, and the epilogue is deferred into the next
head's first chunk, to keep the ACT engine (the bottleneck) fed across
head boundaries.
"""

import numpy as np

N = 4000
D_MODEL = 4096
NUM_HEADS = 32
HD = 128
N_CORES = 8
H_PER_CORE = NUM_HEADS // N_CORES          # 4
D_CORE = H_PER_CORE * HD                   # 512
P = 128
KB = 32                                    # 32 k-blocks of 128 (4096 padded)
NPAD = KB * P                              # 4096
QSUP = 1024                                # q-superchunk (2 PSUM banks)
NQS = NPAD // QSUP                          # 4
PAD_Q = NPAD - N                           # 96 zero-padded q columns
SCALE = 1.0 / np.sqrt(np.float32(HD))

_CACHE = {}

# pT/value dtype for the second matmul. bf16 costs nothing on the PE but
# enables the DVE 4x-mode tensor_scalar accumulate for the softmax
# denominators, taking the per-tile accumulator read off the ACT engine
# (the bottleneck). Set False to fall back to all-fp32r + ACT accum_out.
BF16_PT = True


def _build():
    import concourse.bacc as bacc
    import concourse.tile as tile
    from concourse import mybir
    from concourse import masks

    f32 = mybir.dt.float32
    f32r = mybir.dt.float32r
    bf16 = mybir.dt.bfloat16
    pt_dt = bf16 if BF16_PT else f32r
    Exp = mybir.ActivationFunctionType.Exp
    X = mybir.AxisListType.X
    Add = mybir.AluOpType.add
    Mult = mybir.AluOpType.mult

    nc = bacc.Bacc("TRN2", target_bir_lowering=False, debug=False)
    q_in = nc.declare_dram_parameter("q", [N, D_CORE], f32, isOutput=False)
    o_out = nc.declare_dram_parameter("out", [N, D_CORE], f32, isOutput=True)

    with tile.TileContext(nc) as tc:
        with (
            tc.tile_pool(name="singles", bufs=1) as singles,
            tc.tile_pool(name="qnat", bufs=1) as qnat_pool,
            tc.tile_pool(name="qT", bufs=2) as qT_pool,
            tc.tile_pool(name="vr", bufs=2) as vr_pool,
            tc.tile_pool(name="pt", bufs=4) as pt_pool,
            tc.tile_pool(name="lacc", bufs=2) as lacc_pool,
            tc.tile_pool(name="osb", bufs=2) as osb_pool,
            tc.tile_pool(name="rrec", bufs=2) as r_pool,
            tc.tile_pool(name="obuf", bufs=4) as ob_pool,
            tc.tile_pool(name="ps_s", bufs=2, space="PSUM") as ps_s_pool,
            tc.tile_pool(name="ps_o", bufs=1, space="PSUM") as ps_o_pool,
            tc.tile_pool(name="ps_t", bufs=2, space="PSUM") as ps_t_pool,
        ):
            ident = singles.tile([P, P], f32)
            masks.make_identity(nc, ident)
            # single scratch output for the DVE row-sum pass (value unused;
            # DVE executes in order so the WAW chain costs nothing)
            dummy = singles.tile([P, QSUP], bf16)

            # resident natural-layout q (all 4 heads): 32 tiles [128, 512]
            qnat = []
            for kb in range(KB):
                t = qnat_pool.tile([P, D_CORE], f32, tag=f"qn{kb}")
                if kb == KB - 1:
                    nc.vector.memset(t, 0.0)
                    nc.sync.dma_start(out=t[: N - kb * P, :], in_=q_in[kb * P : N, :])
                else:
                    nc.sync.dma_start(out=t, in_=q_in[kb * P : (kb + 1) * P, :])
                qnat.append(t)

            def build_head_inputs(h):
                """qT [hd, 4096] (f32r, via PE transposes) + value tiles."""
                hs = slice(h * HD, (h + 1) * HD)
                qT = qT_pool.tile([P, NPAD], f32r, tag="qT", name="qT")
                vr = []
                for kb in range(KB):
                    pst = ps_t_pool.tile([P, P], f32, tag="pst", name="pst")
                    nc.tensor.transpose(pst, qnat[kb][:, hs], ident)
                    nc.vector.tensor_copy(qT[:, kb * P : (kb + 1) * P], pst)
                    v_t = vr_pool.tile([P, HD], pt_dt, tag=f"vr{kb}", name=f"vr{kb}")
                    nc.vector.tensor_copy(v_t, qnat[kb][:, hs])
                    vr.append(v_t)
                return qT, vr

            def make_finalize(h, lacc, o_sb):
                hs = slice(h * HD, (h + 1) * HD)

                def finalize():
                    # denominators: l[k] = sum_qs partials ; r = 1/l
                    rt = []
                    for kb in range(KB):
                        r_t = r_pool.tile([P, 2], f32, tag=f"r{kb}", name=f"r{kb}")
                        nc.vector.reduce_sum(r_t[:, 0:1], lacc[kb][:, :], axis=X)
                        if not BF16_PT:
                            nc.vector.tensor_scalar_add(
                                r_t[:, 0:1], r_t[:, 0:1], -float(PAD_Q)
                            )
                        nc.vector.reciprocal(r_t[:, 1:2], r_t[:, 0:1])
                        rt.append(r_t)
                    # transpose O_T back to [q, hd], scale by r, store
                    for qs in range(NQS):
                        for sub in range(QSUP // P):
                            j = qs * (QSUP // P) + sub
                            pst = ps_t_pool.tile([P, P], f32, tag="pst", name="pst")
                            nc.tensor.transpose(
                                pst, o_sb[qs][:, sub * P : (sub + 1) * P], ident
                            )
                            ob = ob_pool.tile([P, P], f32, tag="ob", name="ob")
                            nc.vector.tensor_scalar_mul(ob, pst, rt[j][:, 1:2])
                            rows = P if j < KB - 1 else N - (KB - 1) * P
                            eng = nc.gpsimd if j % 2 == 0 else nc.sync
                            eng.dma_start(
                                out=o_out[j * P : j * P + rows, hs], in_=ob[:rows, :]
                            )

                return finalize

            qT, vr = build_head_inputs(0)
            pending_finalize = None

            def emit_mm1(qT, qs, kb):
                ps_s = ps_s_pool.tile([P, QSUP], f32, tag="ps_s", name="ps_s")
                for hf in range(2):
                    c0 = qs * QSUP + hf * 512
                    nc.tensor.matmul(
                        ps_s[:, hf * 512 : (hf + 1) * 512],
                        lhsT=qT[:, kb * P : (kb + 1) * P],
                        rhs=qT[:, c0 : c0 + 512],
                        start=True,
                        stop=True,
                    )
                return ps_s

            for h in range(H_PER_CORE):
                lacc = [
                    lacc_pool.tile([P, NQS], f32, tag=f"lacc{kb}", name=f"lacc{kb}")
                    for kb in range(KB)
                ]
                o_sb = []
                # mm1 runs one iteration ahead of exp/mm2 so the PE's
                # in-order stream never makes the ACT wait behind mm2
                pending_s = emit_mm1(qT, 0, 0)
                for qs in range(NQS):
                    # valid (non-padded) q columns in this chunk
                    vw = QSUP if qs < NQS - 1 else QSUP - PAD_Q
                    ps_o = ps_o_pool.tile([P, QSUP], f32, tag="ps_o", name="ps_o")
                    for kb in range(KB):
                        ps_s = pending_s
                        if kb + 1 < KB:
                            pending_s = emit_mm1(qT, qs, kb + 1)
                        elif qs + 1 < NQS:
                            pending_s = emit_mm1(qT, qs + 1, 0)
                        pt = pt_pool.tile([P, QSUP], pt_dt, tag="pt", name="pt")
                        if BF16_PT:
                            # exp only the valid columns; the stale tail of the
                            # last chunk feeds mm2 but never reaches the output
                            nc.scalar.activation(
                                pt[:, :vw], ps_s[:, :vw], Exp, scale=float(SCALE)
                            )
                        else:
                            nc.scalar.activation(
                                pt,
                                ps_s,
                                Exp,
                                scale=float(SCALE),
                                accum_out=lacc[kb][:, qs : qs + 1],
                            )
                        for hf in range(2):
                            nc.tensor.matmul(
                                ps_o[:, hf * 512 : (hf + 1) * 512],
                                lhsT=vr[kb],
                                rhs=pt[:, hf * 512 : (hf + 1) * 512],
                                start=(kb == 0),
                                stop=(kb == KB - 1),
                            )
                        if BF16_PT:
                            # softmax denominators on DVE (4x mode, bf16):
                            # row-sum of exp over this chunk's valid columns
                            nc.vector.tensor_scalar(
                                dummy[:, :vw],
                                pt[:, :vw],
                                1.0,
                                None,
                                op0=Mult,
                                op1=Add,
                                accum_out=lacc[kb][:, qs : qs + 1],
                            )
                    osb = osb_pool.tile(
                        [P, QSUP], f32, tag=f"osb{qs}", name=f"osb{qs}"
                    )
                    nc.vector.tensor_copy(osb, ps_o)
                    o_sb.append(osb)
                    if qs == 0 and pending_finalize is not None:
                        # previous head's epilogue lands inside this chunk's
                        # window, where the PE/DVE have slack vs the ACT
                        pending_finalize()
                        pending_finalize = None

                if h + 1 < H_PER_CORE:
                    qT, vr = build_head_inputs(h + 1)
                pending_finalize = make_finalize(h, lacc, o_sb)

            pending_finalize()

    nc.compile()
    return nc


def _get_nc():
    if "nc" not in _CACHE:
        _CACHE["nc"] = _build()
    return _CACHE["nc"]


def kernel(**inputs: np.ndarray) -> np.ndarray:
    from concourse.bass_utils import run_bass_kernel_spmd

    q = np.ascontiguousarray(np.asarray(inputs["q"], dtype=np.float32))
    assert q.shape == (N, D_MODEL)

    nc = _get_nc()
    in_maps = [
        {"q": q[:, c * D_CORE : (c + 1) * D_CORE]} for c in range(N_CORES)
    ]
    res = run_bass_kernel_spmd(nc, in_maps, list(range(N_CORES)))
    out = np.concatenate([res.results[c]["out"] for c in range(N_CORES)], axis=1)
    return out.astype(np.float32)
